# revision 2
# baseline (speedup 1.0000x reference)
"""Trainium2 Bass kernel for nn_Conv2d_24833500905755 (3x3 conv, B=32,
C_in=64, C_out=128, 56x56, pad 1, with the reference's mismatched
weight-flatten order).

Math: out[b,co,h,w] = sum_{c,di,dj} xpad[b,c,h+di,w+dj] * Wt[c,di*3+dj,co]
with Wt = K.reshape(576, C_OUT).reshape(C_IN, 9, C_OUT).

Data-parallel: 4 images per NeuronCore, 2 images packed on the
128-partition dim (fp16 matmuls, K=64 contraction per half, concurrent
PE row-group tiles). Raw-bass hand-scheduled engine programs.

v2 changes vs baseline:
  - real matmul stream starts as soon as the first w/x DMA pieces land
    (previously 28 junk warmups serialized ~6us of half-clock ramp
    before any real work); a small junk prefix only covers the DMA
    ring start latency so the PE p-state ramp begins early.
  - outputs are downcast to fp16 on the PSUM->SBUF copy and DMA'd out
    as fp16 (half the output HBM traffic); host upcasts to fp32.
  - w is DMA'd in two tap-major pieces so tap 0 weights land early.

Engine programs:
  Sync:   pair-0 input DMAs (3 pieces), pair-0 half-0 output DMAs,
          final output-completion wait
  Scalar: w DMAs (2 pieces), pair-1 input DMAs, half-1 PSUM->SBUF
          copies, pair-1 output DMAs
  Tensor: junk ramp prefix + 252 fp16 matmuls, gated on input-piece /
          bank-WAR sems
  Vector: half-0 PSUM->SBUF copies (fp32 -> fp16)
"""

from contextlib import ExitStack

import numpy as np

import concourse.bass as bass
import concourse.mybir as mybir
from concourse.bass_utils import run_bass_kernel_spmd

B, C_IN, C_OUT, H = 32, 64, 128, 56
KS = 3
N_CORES = 8
BPC = B // N_CORES
HP = H + 2
RCHUNK = 8
NCHUNK = H // RCHUNK          # 7 chunks/image, 14 global chunks (2 pairs)
OBLOCKS = [(0, 24), (24, 40), (40, 48), (48, 56)]
MM_DT = mybir.dt.float16
OUT_DT = mybir.dt.float16
N_JUNK = 8                    # ramp prefix; covers DMA ring start latency


def build_nc(mm_dt=MM_DT):
    f32 = mybir.dt.float32
    nc = bass.Bass()
    x_ext = nc.declare_dram_parameter("x", [BPC, C_IN, HP, HP], mm_dt, isOutput=False)
    w_ext = nc.declare_dram_parameter("w", [2 * C_IN, KS * KS, C_OUT], mm_dt, isOutput=False)
    out_ext = nc.declare_dram_parameter("out", [BPC, C_OUT, H, H], OUT_DT, isOutput=True)

    n_out_dmas = 2 * len(OBLOCKS) * 2  # pairs * blocks * halves

    with ExitStack() as ctx:
        wt = ctx.enter_context(nc.sbuf_tensor("wt", [2 * C_IN, KS * KS, C_OUT], mm_dt))
        xps = [
            ctx.enter_context(nc.sbuf_tensor(f"xp{p}", [2 * C_IN, HP, HP], mm_dt))
            for p in range(2)
        ]
        # obs[p][half][block]
        obs = [
            [
                [
                    ctx.enter_context(
                        nc.sbuf_tensor(f"ob_{p}_{h}_{bi}", [C_OUT, bhi - blo, H], OUT_DT)
                    )
                    for bi, (blo, bhi) in enumerate(OBLOCKS)
                ]
                for h in range(2)
            ]
            for p in range(2)
        ]
        # banks[slot][half] - 8 PSUM banks
        banks = [
            [
                ctx.enter_context(
                    nc.psum_tensor(f"ps_{s}_{h}", [C_OUT, RCHUNK, H], f32)
                )
                for h in range(2)
            ]
            for s in range(4)
        ]
        s_w = ctx.enter_context(nc.semaphore("s_w"))
        s_x = [ctx.enter_context(nc.semaphore(f"s_x{p}")) for p in range(2)]
        s_xa = ctx.enter_context(nc.semaphore("s_xa"))
        s_mm = ctx.enter_context(nc.semaphore("s_mm"))
        s_cp = ctx.enter_context(nc.semaphore("s_cp"))
        s_cp2 = ctx.enter_context(nc.semaphore("s_cp2"))
        s_out = ctx.enter_context(nc.semaphore("s_out"))

        with nc.Block() as block:

            @block.sync
            def _(sync: bass.BassEngine):
                src = x_ext[0:2].rearrange("b c h w -> (b c) h w")
                sync.dma_start(out=xps[0][:, 0:10, :], in_=src[:, 0:10, :]).then_inc(s_xa, 16)
                sync.dma_start(out=xps[0][:, 10:34, :], in_=src[:, 10:34, :]).then_inc(s_x[0], 16)
                sync.dma_start(out=xps[0][:, 34:HP, :], in_=src[:, 34:HP, :]).then_inc(s_x[0], 16)
                for p in range(2):
                    for bi, (blo, bhi) in enumerate(OBLOCKS):
                        c_last = p * NCHUNK + (bhi // RCHUNK - 1)
                        sync.wait_ge(s_cp, c_last + 1)
                        dst = out_ext[2 * p : 2 * p + 1].rearrange("b c h w -> (b c) h w")
                        sync.dma_start(
                            out=dst[:, blo:bhi, :], in_=obs[p][0][bi][:]
                        ).then_inc(s_out, 16)
                sync.wait_ge(s_out, 16 * n_out_dmas)

            @block.scalar
            def _(scalar: bass.BassEngine):
                scalar.dma_start(out=wt[:, 0:3, :], in_=w_ext[:, 0:3, :]).then_inc(s_w, 16)
                scalar.dma_start(out=wt[:, 3:, :], in_=w_ext[:, 3:, :]).then_inc(s_w, 16)
                src = x_ext[2:4].rearrange("b c h w -> (b c) h w")
                scalar.dma_start(out=xps[1][:, 0:12, :], in_=src[:, 0:12, :]).then_inc(s_x[1], 16)
                scalar.dma_start(out=xps[1][:, 12:34, :], in_=src[:, 12:34, :]).then_inc(s_x[1], 16)
                scalar.dma_start(out=xps[1][:, 34:HP, :], in_=src[:, 34:HP, :]).then_inc(s_x[1], 16)
                for p in range(2):
                    for ci in range(NCHUNK):
                        c = p * NCHUNK + ci
                        h0 = ci * RCHUNK
                        blo, bhi = next(b for b in OBLOCKS if b[0] <= h0 < b[1])
                        bi = OBLOCKS.index((blo, bhi))
                        scalar.wait_ge(s_mm, 2 * (c + 1))
                        scalar.copy(
                            out=obs[p][1][bi][:, h0 - blo : h0 - blo + RCHUNK, :],
                            in_=banks[c % 4][1][:],
                        ).then_inc(s_cp2, 1)
                        if h0 + RCHUNK == bhi:
                            scalar.wait_ge(s_cp2, c + 1)
                            dst = out_ext[2 * p + 1 : 2 * p + 2].rearrange(
                                "b c h w -> (b c) h w"
                            )
                            scalar.dma_start(
                                out=dst[:, blo:bhi, :], in_=obs[p][1][bi][:]
                            ).then_inc(s_out, 16)

            @block.tensor
            def _(tensor: bass.BassEngine):
                # Junk ramp prefix: start the PE p-state ramp while the first
                # input DMAs are still in flight. banks[3] is first reused by
                # chunk 3 (start=True clears it), well after these complete.
                for wi in range(N_JUNK):
                    h = wi % 2
                    c0 = h * C_IN
                    tensor.matmul(
                        out=banks[3][h][:],
                        lhsT=wt[c0 : c0 + C_IN, 0, :],
                        rhs=xps[0][c0 : c0 + C_IN, 0:RCHUNK, 0:H],
                        start=True,
                        stop=True,
                    )
                for p in range(2):
                    for ci in range(NCHUNK):
                        c = p * NCHUNK + ci
                        h0 = ci * RCHUNK
                        if p == 0:
                            if ci == 0:
                                tensor.wait_ge(s_w, 16)
                                tensor.wait_ge(s_xa, 16)  # rows [0,10)
                            elif ci == 1:
                                tensor.wait_ge(s_x[0], 16)  # rows [10,34)
                            elif ci == 4:
                                tensor.wait_ge(s_x[0], 32)  # rows [34,58)
                        else:
                            if ci == 0:
                                tensor.wait_ge(s_x[1], 16)
                            elif ci == 1:
                                tensor.wait_ge(s_x[1], 32)
                            elif ci == 4:
                                tensor.wait_ge(s_x[1], 48)
                        if c >= 4:
                            # WAR: bank slot c%4 last used by chunk c-4
                            tensor.wait_ge(s_cp, c - 3)
                            tensor.wait_ge(s_cp2, c - 3)
                        for k in range(KS * KS):
                            di, dj = divmod(k, KS)
                            last = k == KS * KS - 1
                            if p == 0 and ci == 0 and k == 3:
                                tensor.wait_ge(s_w, 32)  # taps 3-8
                            for half in range(2):
                                c0 = half * C_IN
                                mm = tensor.matmul(
                                    out=banks[c % 4][half][:],
                                    lhsT=wt[c0 : c0 + C_IN, k, :],
                                    rhs=xps[p][
                                        c0 : c0 + C_IN,
                                        h0 + di : h0 + di + RCHUNK,
                                        dj : dj + H,
                                    ],
                                    start=(k == 0),
                                    stop=last,
                                )
                                if last and half == 1:
                                    mm.then_inc(s_mm, 2)

            @block.vector
            def _(vector: bass.BassEngine):
                for p in range(2):
                    for ci in range(NCHUNK):
                        c = p * NCHUNK + ci
                        h0 = ci * RCHUNK
                        blo, bhi = next(b for b in OBLOCKS if b[0] <= h0 < b[1])
                        bi = OBLOCKS.index((blo, bhi))
                        vector.wait_ge(s_mm, 2 * (c + 1))
                        vector.tensor_copy(
                            out=obs[p][0][bi][:, h0 - blo : h0 - blo + RCHUNK, :],
                            in_=banks[c % 4][0][:],
                        ).then_inc(s_cp, 1)

    return nc


def _prep_inputs(x, K, mm_dt=MM_DT):
    np_dt = mybir.dt.np(mm_dt)
    x = np.ascontiguousarray(np.asarray(x, dtype=np.float32))
    K = np.ascontiguousarray(np.asarray(K, dtype=np.float32))
    xpad = np.pad(x, ((0, 0), (0, 0), (1, 1), (1, 1))).astype(np_dt)
    Wt = K.reshape(KS * KS * C_IN, C_OUT).reshape(C_IN, KS * KS, C_OUT)
    Wrep = np.ascontiguousarray(np.concatenate([Wt, Wt], axis=0)).astype(np_dt)
    shards = xpad.reshape(N_CORES, BPC, C_IN, HP, HP)
    return [{"x": np.ascontiguousarray(shards[i]), "w": Wrep} for i in range(N_CORES)]


def run(x, K, trace=False, mm_dt=MM_DT):
    nc = build_nc(mm_dt)
    in_maps = _prep_inputs(x, K, mm_dt)
    res = run_bass_kernel_spmd(nc, in_maps, list(range(N_CORES)), trace=trace)
    out = np.concatenate([res.results[i]["out"] for i in range(N_CORES)], axis=0)
    return out.astype(np.float32), res


def kernel(x, K):
    out, _ = run(x, K, trace=False)
    return out


# revision 3
# speedup vs baseline: 1.0795x; 1.0795x over previous
"""Trainium2 Bass kernel for nn_Conv2d_24833500905755 (3x3 conv, B=32,
C_in=64, C_out=128, 56x56, pad 1, with the reference's mismatched
weight-flatten order).

Math: out[b,co,h,w] = sum_{c,di,dj} xpad[b,c,h+di,w+dj] * Wt[c,di*3+dj,co]
with Wt = K.reshape(576, C_OUT).reshape(C_IN, 9, C_OUT).

Data-parallel: 4 images per NeuronCore, 2 images packed on the
128-partition dim (fp16 matmuls, K=64 contraction per half, concurrent
PE row-group tiles). Raw-bass hand-scheduled engine programs.

v3 scheduling model (calibrated from perfetto traces):
  - DMA instruction latency ~2.2us (hwdge 0.63 + dge 0.65 + sem 0.9)
    plus transfer at 360 GB/s aggregate; concurrent DMAs share the 16
    queues fairly, so a critical piece must not compete with bulk.
  - PE p-state ramp: half clock for ~6us from the first matmul, and it
    RESETS if the PE goes idle -> the PE must run continuously.
  - Junk matmuls increment s_junk, which other engines use as a clock
    to stagger DMA issues without paying completion-gating latency.

Engine programs:
  Sync:   xA (rows 0:10) at t0; xB at s_junk>=2; xC at s_mm>=2;
          pair-0 half-0 output DMAs; final output-completion wait
  Scalar: w (single DMA) at t0; half-1 PSUM->SBUF copies (fp32->fp16),
          pair-1 output DMAs
  GpSimd: pair-1 input DMAs at s_mm>=4
  Tensor: junk ramp prefix (clocked), then 252 fp16 matmuls
  Vector: half-0 PSUM->SBUF copies (fp32 -> fp16)

Output is fp16 on-chip and in HBM; host upcasts to fp32.
"""

from contextlib import ExitStack

import numpy as np

import concourse.bass as bass
import concourse.mybir as mybir
from concourse.bass_utils import run_bass_kernel_spmd

B, C_IN, C_OUT, H = 32, 64, 128, 56
KS = 3
N_CORES = 8
BPC = B // N_CORES
HP = H + 2
RCHUNK = 8
NCHUNK = H // RCHUNK          # 7 chunks/image, 14 global chunks (2 pairs)
OBLOCKS = [(0, 24), (24, 40), (40, 48), (48, 56)]
MM_DT = mybir.dt.float16
OUT_DT = mybir.dt.float16
N_JUNK = 6                    # 448-col junks; ramp prefix ends ~9.8-10.0us
N_JUNK_SMALL = 2              # 112-col tail junks for granularity


def build_nc(mm_dt=MM_DT):
    f32 = mybir.dt.float32
    nc = bass.Bass()
    x_ext = nc.declare_dram_parameter("x", [BPC, C_IN, HP, HP], mm_dt, isOutput=False)
    w_ext = nc.declare_dram_parameter("w", [2 * C_IN, KS * KS, C_OUT], mm_dt, isOutput=False)
    out_ext = nc.declare_dram_parameter("out", [BPC, C_OUT, H, H], OUT_DT, isOutput=True)

    n_out_dmas = 2 * len(OBLOCKS) * 2  # pairs * blocks * halves

    with ExitStack() as ctx:
        wt = ctx.enter_context(nc.sbuf_tensor("wt", [2 * C_IN, KS * KS, C_OUT], mm_dt))
        xps = [
            ctx.enter_context(nc.sbuf_tensor(f"xp{p}", [2 * C_IN, HP, HP], mm_dt))
            for p in range(2)
        ]
        # obs[p][half][block]
        obs = [
            [
                [
                    ctx.enter_context(
                        nc.sbuf_tensor(f"ob_{p}_{h}_{bi}", [C_OUT, bhi - blo, H], OUT_DT)
                    )
                    for bi, (blo, bhi) in enumerate(OBLOCKS)
                ]
                for h in range(2)
            ]
            for p in range(2)
        ]
        # banks[slot][half] - 8 PSUM banks
        banks = [
            [
                ctx.enter_context(
                    nc.psum_tensor(f"ps_{s}_{h}", [C_OUT, RCHUNK, H], f32)
                )
                for h in range(2)
            ]
            for s in range(4)
        ]
        s_w = ctx.enter_context(nc.semaphore("s_w"))
        s_x = [ctx.enter_context(nc.semaphore(f"s_x{p}")) for p in range(2)]
        s_xa = ctx.enter_context(nc.semaphore("s_xa"))
        s_junk = ctx.enter_context(nc.semaphore("s_junk"))
        s_mm = ctx.enter_context(nc.semaphore("s_mm"))
        s_cp = ctx.enter_context(nc.semaphore("s_cp"))
        s_cp2 = ctx.enter_context(nc.semaphore("s_cp2"))
        s_out = ctx.enter_context(nc.semaphore("s_out"))

        with nc.Block() as block:

            @block.sync
            def _(sync: bass.BassEngine):
                src = x_ext[0:2].rearrange("b c h w -> (b c) h w")
                # xA: rows for chunk 0; issued immediately, shares queues
                # only with w (436KB total -> sem ~9.5us)
                sync.dma_start(out=xps[0][:, 0:10, :], in_=src[:, 0:10, :]).then_inc(s_xa, 16)
                # xB: rows for chunks 1-3; staggered so xA gets the queues
                # to itself (junk clock tick 2 ~ 8.3us, xA wire-done ~8.6)
                sync.wait_ge(s_junk, 2)
                sync.dma_start(out=xps[0][:, 10:34, :], in_=src[:, 10:34, :]).then_inc(s_x[0], 16)
                # xC: rows for chunks 4-6; after chunk 0's matmuls are done
                sync.wait_ge(s_mm, 2)
                sync.dma_start(out=xps[0][:, 34:HP, :], in_=src[:, 34:HP, :]).then_inc(s_x[0], 16)
                for p in range(2):
                    for bi, (blo, bhi) in enumerate(OBLOCKS):
                        c_last = p * NCHUNK + (bhi // RCHUNK - 1)
                        sync.wait_ge(s_cp, c_last + 1)
                        dst = out_ext[2 * p : 2 * p + 1].rearrange("b c h w -> (b c) h w")
                        sync.dma_start(
                            out=dst[:, blo:bhi, :], in_=obs[p][0][bi][:]
                        ).then_inc(s_out, 16)
                sync.wait_ge(s_out, 16 * n_out_dmas)

            @block.scalar
            def _(scalar: bass.BassEngine):
                scalar.dma_start(out=wt[:], in_=w_ext[:]).then_inc(s_w, 16)
                for p in range(2):
                    for ci in range(NCHUNK):
                        c = p * NCHUNK + ci
                        h0 = ci * RCHUNK
                        blo, bhi = next(b for b in OBLOCKS if b[0] <= h0 < b[1])
                        bi = OBLOCKS.index((blo, bhi))
                        scalar.wait_ge(s_mm, 2 * (c + 1))
                        scalar.copy(
                            out=obs[p][1][bi][:, h0 - blo : h0 - blo + RCHUNK, :],
                            in_=banks[c % 4][1][:],
                        ).then_inc(s_cp2, 1)
                        if h0 + RCHUNK == bhi:
                            scalar.wait_ge(s_cp2, c + 1)
                            dst = out_ext[2 * p + 1 : 2 * p + 2].rearrange(
                                "b c h w -> (b c) h w"
                            )
                            scalar.dma_start(
                                out=dst[:, blo:bhi, :], in_=obs[p][1][bi][:]
                            ).then_inc(s_out, 16)

            @block.gpsimd
            def _(gpsimd: bass.BassEngine):
                # pair-1 inputs: late enough to not compete with critical
                # pieces, early enough for chunk 7 (~23us)
                gpsimd.wait_ge(s_mm, 4)
                src = x_ext[2:4].rearrange("b c h w -> (b c) h w")
                gpsimd.dma_start(out=xps[1][:, 0:12, :], in_=src[:, 0:12, :]).then_inc(s_x[1], 16)
                gpsimd.dma_start(out=xps[1][:, 12:34, :], in_=src[:, 12:34, :]).then_inc(s_x[1], 16)
                gpsimd.dma_start(out=xps[1][:, 34:HP, :], in_=src[:, 34:HP, :]).then_inc(s_x[1], 16)

            @block.tensor
            def _(tensor: bass.BassEngine):
                # Junk ramp prefix: starts the PE p-state ramp while the
                # first input DMAs are in flight; each junk ticks s_junk so
                # other engines can stagger DMA issues off the PE clock.
                # banks[3] is first reused by chunk 3 (start=True clears it).
                for wi in range(N_JUNK):
                    h = wi % 2
                    c0 = h * C_IN
                    tensor.matmul(
                        out=banks[3][h][:],
                        lhsT=wt[c0 : c0 + C_IN, 0, :],
                        rhs=xps[0][c0 : c0 + C_IN, 0:RCHUNK, 0:H],
                        start=True,
                        stop=True,
                    ).then_inc(s_junk, 1)
                for wi in range(N_JUNK_SMALL):
                    tensor.matmul(
                        out=banks[3][0][:, 0:2, :],
                        lhsT=wt[0:C_IN, 0, :],
                        rhs=xps[0][0:C_IN, 0:2, 0:H],
                        start=True,
                        stop=True,
                    )
                for p in range(2):
                    for ci in range(NCHUNK):
                        c = p * NCHUNK + ci
                        h0 = ci * RCHUNK
                        if p == 0:
                            if ci == 0:
                                tensor.wait_ge(s_w, 16)
                                tensor.wait_ge(s_xa, 16)  # rows [0,10)
                            elif ci == 1:
                                tensor.wait_ge(s_x[0], 16)  # rows [10,34)
                            elif ci == 4:
                                tensor.wait_ge(s_x[0], 32)  # rows [34,58)
                        else:
                            if ci == 0:
                                tensor.wait_ge(s_x[1], 16)
                            elif ci == 1:
                                tensor.wait_ge(s_x[1], 32)
                            elif ci == 4:
                                tensor.wait_ge(s_x[1], 48)
                        if c >= 4:
                            # WAR: bank slot c%4 last used by chunk c-4
                            tensor.wait_ge(s_cp, c - 3)
                            tensor.wait_ge(s_cp2, c - 3)
                        for k in range(KS * KS):
                            di, dj = divmod(k, KS)
                            last = k == KS * KS - 1
                            for half in range(2):
                                c0 = half * C_IN
                                mm = tensor.matmul(
                                    out=banks[c % 4][half][:],
                                    lhsT=wt[c0 : c0 + C_IN, k, :],
                                    rhs=xps[p][
                                        c0 : c0 + C_IN,
                                        h0 + di : h0 + di + RCHUNK,
                                        dj : dj + H,
                                    ],
                                    start=(k == 0),
                                    stop=last,
                                )
                                if last and half == 1:
                                    mm.then_inc(s_mm, 2)

            @block.vector
            def _(vector: bass.BassEngine):
                for p in range(2):
                    for ci in range(NCHUNK):
                        c = p * NCHUNK + ci
                        h0 = ci * RCHUNK
                        blo, bhi = next(b for b in OBLOCKS if b[0] <= h0 < b[1])
                        bi = OBLOCKS.index((blo, bhi))
                        vector.wait_ge(s_mm, 2 * (c + 1))
                        vector.tensor_copy(
                            out=obs[p][0][bi][:, h0 - blo : h0 - blo + RCHUNK, :],
                            in_=banks[c % 4][0][:],
                        ).then_inc(s_cp, 1)

    return nc


def _prep_inputs(x, K, mm_dt=MM_DT):
    np_dt = mybir.dt.np(mm_dt)
    x = np.ascontiguousarray(np.asarray(x, dtype=np.float32))
    K = np.ascontiguousarray(np.asarray(K, dtype=np.float32))
    xpad = np.pad(x, ((0, 0), (0, 0), (1, 1), (1, 1))).astype(np_dt)
    Wt = K.reshape(KS * KS * C_IN, C_OUT).reshape(C_IN, KS * KS, C_OUT)
    Wrep = np.ascontiguousarray(np.concatenate([Wt, Wt], axis=0)).astype(np_dt)
    shards = xpad.reshape(N_CORES, BPC, C_IN, HP, HP)
    return [{"x": np.ascontiguousarray(shards[i]), "w": Wrep} for i in range(N_CORES)]


def run(x, K, trace=False, mm_dt=MM_DT):
    nc = build_nc(mm_dt)
    in_maps = _prep_inputs(x, K, mm_dt)
    res = run_bass_kernel_spmd(nc, in_maps, list(range(N_CORES)), trace=trace)
    out = np.concatenate([res.results[i]["out"] for i in range(N_CORES)], axis=0)
    return out.astype(np.float32), res


def kernel(x, K):
    out, _ = run(x, K, trace=False)
    return out
